# revision 18
# baseline (speedup 1.0000x reference)
"""ExpertConv2d Trainium2 kernel: per-patch mixture-of-experts 3x3 conv.

Problem: x (4,64,512,512) f32 split into 256 patches of (64ch, 64x64);
each patch convolved (pad=1) with a per-patch mix of 5 expert kernels
(mix weights v), plus mixed bias.  Data-parallel over patches across 8
NeuronCores (32 patches/core, processed as 16 patch-pairs).

Device plan per core:
 - warmup: a few junk matmuls during the initial weight DMAs keep the PE
   HAM activity monitor busy so mixing runs at 2.4 GHz, not 1.2.
 - mixing: agg[p, f] = sum_k vv[p,k] * W_flat[k, f] via 4 concurrent
   32x32 PE tiles (tile_position diag (32s,32s)); strip s holds f-quarter
   s of the weights (+ a copy of vv) on partitions 32s..32s+4.  18 waves
   of 4 matmuls, each wave drained by ONE [128,512] psum->sbuf copy into
   agg4 (agg4[32s+p, f'] = agg[p, 9216 s + f']).
 - bias: computed on DVE (no PSUM) as sum_k BBH[:,k] * VBH_k — an
   outer-product accumulation in the [128 part(co,half), 32 (patch)]
   layout the conv copy consumes directly.
 - redistribute: per patch one SB->SB DMA from agg4's 4 strips into the
   per-pair weight tile w_all [128, npair, 9*64] (patch A partitions
   0-63, B 64-127); first 2 pairs on the HWDGE queues, the rest stream
   on gpsimd (SWDGE sustains ~0.7us/DMA without blocking ACT).
 - conv: per pair, x tile [128, 4096] bf16 (A | B channel blocks).  Per
   chunk (8 y-rows = 512 outputs) 9 tap-matmuls accumulate in PSUM;
   boundary taps shrink the output rectangle.  Quadrants: row group =
   patch half, col group (psum half) = patch ^ chunk parity, so 4
   K=64/M=64 matmuls run concurrently = full PE at 215ns/wave.
 - PSUM banks are pinned: chunk c always lives in bank c (tag pc{c}),
   so pair j+1's accumulation only waits on pair j's same-bank copy,
   ~7.5us earlier.  The scalar engine issues no DMAs during conv so the
   ACT bias-add copies never lag the PE.
 - copyback: ACT/DVE per-partition bias add PSUM->SBUF, then per
   half-pair 0.5MB DMAs out (scalar queue; last pairs split across
   sync+scalar quarters to shorten the tail).  Host unscrambles.
"""

import os
import sys

import numpy as np

sys.path.insert(0, "/opt/trn_rl_repo")

import concourse.bass as bass  # noqa: E402
import concourse.tile as tile  # noqa: E402
from concourse import mybir  # noqa: E402

import bass_rust as _bass_rust  # noqa: E402

# ---------------------------------------------------------------------------
# Workaround: this walrus build rejects >2 sync-waits on one instruction.
# TileContext._drain_and_barrier attaches one wait per live sem lane to a
# single SP Drain.  Replace it: one SP wait_ge per lane, then a clean drain.
# ---------------------------------------------------------------------------


def _split_drain_and_barrier(self, tick_clock, wait_clock):
    nc = self.nc
    gc = tick_clock.global_clock
    assert self.sems is not None
    allocated = self.sems.allocated()
    for proc, sem in sorted(allocated.items()):
        t = gc[proc] if proc < len(gc) else 0
        if t > 0:
            nc.sync.wait_ge(sem, _bass_rust.tick_to_sem(t, proc))
    nc.sync.drain()
    nc.all_engine_barrier()
    popped = nc._tile_sem_poison_stack.pop()
    assert popped is self._sem_poison
    nc.clear_and_free_semaphores(list(allocated.values()))
    nc.all_engine_barrier()


tile.TileContext._drain_and_barrier = _split_drain_and_barrier

_MAX_WAITS = 1


def _split_excess_waits(nc):
    """Walrus (CoreV2/V3 setupSyncWait) accepts at most 2 sem-waits per
    instruction.  Tile can attach more.  Move the excess onto NoOps inserted
    immediately before the instruction on the same engine (same queue order,
    so semantics are unchanged)."""
    n_split = 0
    for fn in nc.m.functions:
        for bb in fn.blocks:
            insts = list(bb.instructions)
            out = []
            changed = False
            for inst in insts:
                si = inst.sync_info
                waits = list(si.on_wait) if si is not None and si.on_wait else []
                if len(waits) > _MAX_WAITS:
                    keep = waits[-_MAX_WAITS:]
                    excess = waits[:-_MAX_WAITS]
                    for i in range(0, len(excess), _MAX_WAITS):
                        grp = excess[i:i + _MAX_WAITS]
                        nop = mybir.InstNoOp(
                            name=f"{inst.name}_wsplit{i}", ins=[], outs=[])
                        nop.engine = inst.engine
                        nop.sync_info = mybir.SyncInfo(on_wait=grp, on_update=[])
                        out.append(nop)
                    inst.sync_info = mybir.SyncInfo(
                        on_wait=keep,
                        on_update=list(si.on_update) if si.on_update else [])
                    changed = True
                    n_split += 1
                out.append(inst)
            if changed:
                bb.instructions = out
    return n_split


def _ldw_sig(inst):
    a = inst.ins[0]
    return (getattr(a, 'memref', None), getattr(a, 'offset', None),
            str(getattr(a, 'ap', None)), str(getattr(a, 'dtype', None)))


def _strip_reuse_ldweights(nc, reuse_names):
    """Remove InstLdweights that are provably redundant: the SAME weights
    (same AP signature) are already loaded in the SAME PE quadrant by an
    earlier surviving ldweights in final stream order, with no overlapping
    load in between.  The tile scheduler may reorder the per-tap matmul
    waves (e.g. when PSUM-bank WAR waits differ), so adjacency in the
    emission order is NOT sufficient — track actual loaded state per
    quadrant rectangle.  Merges the stripped ldweights' sync info into the
    matmul so no ordering edges are lost."""
    n = 0
    for fn in nc.m.functions:
        for bb in fn.blocks:
            insts = list(bb.instructions)
            out = []
            changed = False
            loaded = {}   # (row, col) -> (tile_size, sig)
            k = 0
            while k < len(insts):
                inst = insts[k]
                nxt = insts[k + 1] if k + 1 < len(insts) else None
                if isinstance(inst, mybir.InstLdweights):
                    pos = tuple(inst.tile_position) if inst.tile_position \
                        else (0, 0)
                    size = tuple(inst.tile_size) if inst.tile_size \
                        else (128, 128)
                    sig = _ldw_sig(inst)
                    prev = loaded.get(pos)
                    if (nxt is not None and isinstance(nxt, mybir.InstMatmult)
                            and nxt.name in reuse_names
                            and prev is not None and prev == (size, sig)):
                        lsi = inst.sync_info
                        if lsi is not None and (lsi.on_wait or lsi.on_update):
                            msi = nxt.sync_info
                            mw = list(msi.on_wait) if msi and msi.on_wait else []
                            mu = list(msi.on_update) if msi and msi.on_update else []
                            nxt.sync_info = mybir.SyncInfo(
                                on_wait=list(lsi.on_wait or []) + mw,
                                on_update=mu + list(lsi.on_update or []))
                        changed = True
                        n += 1
                        k += 1
                        continue
                    # this load executes: invalidate every tracked rectangle
                    # it overlaps, then record it
                    r0, c0 = pos
                    r1, c1 = r0 + size[0], c0 + size[1]
                    for opos in list(loaded):
                        osz = loaded[opos][0]
                        if (opos[0] < r1 and r0 < opos[0] + osz[0]
                                and opos[1] < c1 and c0 < opos[1] + osz[1]):
                            del loaded[opos]
                    loaded[pos] = (size, sig)
                out.append(inst)
                k += 1
            if changed:
                bb.instructions = out
    return n


# ---------------------------------------------------------------------------
# Constants (hardcoded problem shape)
# ---------------------------------------------------------------------------
B, C_IN, C_OUT, K, KS, P_SZ, HW = 4, 64, 64, 5, 3, 64, 512
GRID = HW // P_SZ                  # 8x8 patch grid
N_CORES = 8
N_PATCH = B * GRID * GRID          # 256
PPC = N_PATCH // N_CORES           # 32 patches per core
NPAIR = PPC // 2                   # 16 pairs per core
NCHUNK = 8                         # 512-wide output chunks per patch
WFREE = C_IN * KS * KS * C_OUT     # 36864  (ci, t, co) flat weight size
QF = WFREE // 4                    # 9216   f-quarter per mixing strip
NWAVE = QF // 512                  # 18 mixing waves
SCOLS = QF + PPC                   # 9248   strip cols: w quarter + vv block
N_WARM = 14                        # junk matmuls to keep HAM busy pre-mix
BF16 = mybir.dt.bfloat16
F32 = mybir.dt.float32

_NC_CACHE = {}
_REUSE_MM_NAMES = set()


def _tap_geometry(c, ky, kx):
    """Output sub-rectangle of chunk c covered by tap (ky, kx) and the
    matching input offset.  Returns None if empty (never happens here)."""
    y0 = max(8 * c, 1 - ky)
    y1 = min(8 * c + 8, P_SZ + 1 - ky)
    x0 = max(0, 1 - kx)
    x1 = min(P_SZ, P_SZ + 1 - kx)
    if y0 >= y1 or x0 >= x1:
        return None
    in_off = (y0 + ky - 1) * P_SZ + (x0 + kx - 1)
    out_off = (y0 - 8 * c) * P_SZ + x0
    return in_off, out_off, y1 - y0, x1 - x0


def build_nc(npair=NPAIR, split_waits=True):
    nc = bass.Bass("TRN2")
    xin = nc.dram_tensor("xin", [npair, 128, P_SZ * P_SZ], BF16, kind="ExternalInput")
    # wrepv[s, k, 0:QF] = wflat[k, QF*s : QF*(s+1)]; [.., QF:SCOLS] = vv[k, :]
    wrepv = nc.dram_tensor("wrepv", [4, K, SCOLS], BF16, kind="ExternalInput")
    # biasin[:, 32k+p] = VBH_k[q, p]; [:, 160+k] = BBH[q, k]  (see marshal)
    biasin = nc.dram_tensor("biasin", [128, PPC * K + K], F32, kind="ExternalInput")
    out = nc.dram_tensor("out", [npair, 128, P_SZ * P_SZ], BF16, kind="ExternalOutput")

    npatch = 2 * npair
    with tile.TileContext(nc) as tc:
        with (
            tc.tile_pool(name="persist", bufs=1) as persist,
            tc.tile_pool(name="xpool", bufs=5) as xpool,
            tc.tile_pool(name="opool", bufs=4) as opool,
            tc.tile_pool(name="psum", bufs=1, space="PSUM") as pp,
        ):
            # ---- constant loads spread over the three DMA queues ----
            wv_sb = persist.tile([128, SCOLS], BF16)
            nc.sync.dma_start(out=wv_sb[0:K, :], in_=wrepv[0, :, :])
            nc.scalar.dma_start(out=wv_sb[64:64 + K, :], in_=wrepv[2, :, :])
            nc.sync.dma_start(out=wv_sb[32:32 + K, :], in_=wrepv[1, :, :])
            nc.scalar.dma_start(out=wv_sb[96:96 + K, :], in_=wrepv[3, :, :])
            bin_sb = persist.tile([128, PPC * K + K], F32)
            nc.gpsimd.dma_start(out=bin_sb, in_=biasin[:, :])

            # ---- prefetch first pairs' x ----
            x_pre = {}
            x_pre[0] = xpool.tile([128, P_SZ * P_SZ], BF16, tag="x", name="x_t")
            nc.scalar.dma_start(out=x_pre[0], in_=xin[0, :, :])
            x_pre[1] = xpool.tile([128, P_SZ * P_SZ], BF16, tag="x", name="x_t")
            nc.sync.dma_start(out=x_pre[1], in_=xin[1, :, :])

            # ---- PE warmup: junk matmuls on a memset tile while the
            #      weight DMAs are in flight (keeps HAM at 2.4 GHz) ----
            warm_sb = persist.tile([128, 512], BF16)
            nc.vector.memset(warm_sb, 0.0)
            for i in range(N_WARM):
                pw = pp.tile([128, 512], F32, tag=f"pc{5 + i % 3}", name="pw")
                nc.tensor.matmul(pw, lhsT=warm_sb[:, 0:128], rhs=warm_sb,
                                 start=True, stop=True, skip_group_check=True)

            # ---- weight mixing: 4 concurrent 32x32 PE tiles (diag),
            #      strip s computes f-quarter s for all 32 patches ----
            agg4_sb = persist.tile([128, QF], BF16)
            for w in range(NWAVE):
                pm = pp.tile([128, 512], F32, tag=f"pc{w % 8}", name="pm")
                for s in range(4):
                    nc.tensor.matmul(
                        pm[32 * s:32 * s + 32, :],
                        lhsT=wv_sb[32 * s:32 * s + K, QF:QF + PPC],
                        rhs=wv_sb[32 * s:32 * s + K, 512 * w:512 * (w + 1)],
                        start=True, stop=True, skip_group_check=True,
                        tile_position=(32 * s, 32 * s))
                dst = agg4_sb[:, 512 * w:512 * (w + 1)]
                if w % 2 == 0:
                    nc.scalar.copy(out=dst, in_=pm)
                else:
                    nc.vector.tensor_copy(dst, pm)

            # ---- per-pair weight tiles: [128, 9*64] bf16;
            #      partitions 0-63 patch A taps, 64-127 patch B.
            #      agg4 strip s holds ci 16s..16s+16 -> partition = ci. ----
            w_all = persist.tile([128, npair, KS * KS * C_OUT], BF16)
            for p in range(npatch):
                j, hp = p // 2, p % 2
                src = agg4_sb[p::32, :].rearrange("s (ci f) -> s ci f", ci=16)
                eng = (nc.sync, nc.scalar)[p] if p < 2 else nc.gpsimd
                eng.dma_start(out=w_all[64 * hp:64 * hp + 64, j, :], in_=src)

            # ---- bias on DVE (PSUM-free):
            #      bias_sb[q, p] = sum_k BBH[q, k] * VBH[q, 32k+p] ----
            bias_sb = persist.tile([128, npatch], F32)
            bias_tmp = persist.tile([128, npatch], F32)
            acc, other = bias_sb, bias_tmp
            nc.vector.tensor_scalar_mul(acc, bin_sb[:, 0:PPC],
                                        bin_sb[:, PPC * K:PPC * K + 1])
            for k in range(1, K):
                acc, other = other, acc
                nc.vector.scalar_tensor_tensor(
                    out=acc, in0=bin_sb[:, PPC * k:PPC * (k + 1)],
                    scalar=bin_sb[:, PPC * K + k:PPC * K + k + 1],
                    in1=other, op0=mybir.AluOpType.mult,
                    op1=mybir.AluOpType.add)
            assert acc is bias_sb  # K odd: final result lands in bias_sb

            # ---- main loop over pairs ----
            taps = [(1, 1)] + [(ky, kx) for ky in range(KS) for kx in range(KS)
                               if (ky, kx) != (1, 1)]
            for j in range(npair):
                if j in x_pre:
                    x_t = x_pre[j]
                else:
                    x_t = xpool.tile([128, P_SZ * P_SZ], BF16, tag="x")
                    nc.sync.dma_start(out=x_t, in_=xin[j, :, :])
                o_t = opool.tile([128, P_SZ * P_SZ], BF16, tag="o")

                for c4 in range(NCHUNK // 4):
                    chunks = tuple(4 * c4 + i for i in range(4))
                    psums = {}
                    for c in chunks:
                        psums[c] = pp.tile([128, 512], F32, tag=f"pc{c}",
                                           name=f"pc{c}")
                    for ti, (ky, kx) in enumerate(taps):
                        first = ti == 0
                        last = ti == len(taps) - 1
                        # Order so the 4 in-flight matmuls cover 4 distinct
                        # PSUM banks and all 4 PE quadrants; second wave reuses
                        # each quadrant's already-loaded weights.
                        order = [(chunks[0], 0, False), (chunks[1], 0, False),
                                 (chunks[2], 1, False), (chunks[3], 1, False),
                                 (chunks[2], 0, True), (chunks[3], 0, True),
                                 (chunks[0], 1, True), (chunks[1], 1, True)]
                        for c, P, reuse in order:
                            h = P ^ (c & 1)
                            in_off, out_off, cy, cx = _tap_geometry(c, ky, kx)
                            y_in0 = in_off // P_SZ
                            x_in0 = in_off % P_SZ
                            rhs = x_t[64 * P:64 * P + 64, :].rearrange(
                                "p (y x) -> p y x", x=P_SZ)[
                                :, y_in0:y_in0 + cy, x_in0:x_in0 + cx]
                            y_o0 = out_off // P_SZ
                            x_o0 = out_off % P_SZ
                            outap = psums[c][64 * h:64 * h + 64, :].rearrange(
                                "p (y x) -> p y x", x=P_SZ)[
                                :, y_o0:y_o0 + cy, x_o0:x_o0 + cx]
                            t = ky * KS + kx
                            lhsT = w_all[64 * P:64 * P + 64, j,
                                         t * C_OUT:(t + 1) * C_OUT]
                            mi = nc.tensor.matmul(outap, lhsT=lhsT, rhs=rhs,
                                                  start=first, stop=last,
                                                  skip_group_check=True)
                            if reuse:
                                _REUSE_MM_NAMES.add(mi.ins.name)
                    for c in chunks:
                        col = 2 * j + (c & 1)
                        dst = o_t[:, c * 512:(c + 1) * 512]
                        if c & 1:
                            nc.vector.tensor_scalar_add(
                                dst, psums[c], bias_sb[:, col:col + 1])
                        else:
                            nc.scalar.activation(
                                dst, psums[c],
                                mybir.ActivationFunctionType.Identity,
                                bias=bias_sb[:, col:col + 1], scale=1.0)
                    # stream this half-pair out as soon as its copies land
                    half_f = P_SZ * P_SZ // 2
                    f0 = c4 * half_f
                    if j < npair - 2:
                        nc.scalar.dma_start(out=out[j, :, f0:f0 + half_f],
                                            in_=o_t[:, f0:f0 + half_f])
                    else:
                        qf = half_f // 2
                        for q in range(2):
                            eng = nc.sync if q == 0 else nc.scalar
                            a = f0 + q * qf
                            eng.dma_start(out=out[j, :, a:a + qf],
                                          in_=o_t[:, a:a + qf])
    ns = _strip_reuse_ldweights(nc, _REUSE_MM_NAMES)
    if split_waits:
        n = _split_excess_waits(nc)
        if n:
            print(f"[kernel] split {n} waits; stripped {ns} ldweights")
    return nc


# ---------------------------------------------------------------------------
# Host marshalling
# ---------------------------------------------------------------------------


def _marshal_inputs(x, v, weight, bias):
    import ml_dtypes

    bf16 = ml_dtypes.bfloat16
    # x: (B, C, 512, 512) -> per patch (b, gy, gx) blocks of [64, 64, 64]
    xp = x.reshape(B, C_IN, GRID, P_SZ, GRID, P_SZ)
    xp = xp.transpose(0, 2, 4, 1, 3, 5)          # b gy gx ci y x
    xp = np.ascontiguousarray(xp).reshape(N_PATCH, C_IN, P_SZ * P_SZ)
    # per core: [NPAIR, 128(=2 patches x ci), 4096]
    xc = xp.reshape(N_CORES, NPAIR, 2 * C_IN, P_SZ * P_SZ)

    # vv: (b, k, gy, gx) -> [patch, k]
    vv = v.transpose(0, 2, 3, 1).reshape(N_PATCH, K)
    vvc = vv.reshape(N_CORES, PPC, K)            # [core, p, k]
    vv_lo = vvc.transpose(0, 2, 1)               # [core, K, 32]

    # weight: (k, co, ci, ky, kx) -> [k, (ci, t, co)] -> strips + vv block
    wf = weight.transpose(0, 2, 3, 4, 1).reshape(K, C_IN, KS * KS, C_OUT)
    wf = np.ascontiguousarray(wf).reshape(K, WFREE)
    wq = wf.reshape(K, 4, QF).transpose(1, 0, 2)          # [4, K, QF]

    # bias DVE inputs: VBH_k[q, p] = vv[p_eff, k] (p_eff pair-swapped for
    # q >= 64), BBH[q, k] = bias[k, q % 64]
    swap = vvc.reshape(N_CORES, NPAIR, 2, K)[:, :, ::-1, :].reshape(
        N_CORES, PPC, K)                          # [core, p, k] pair-swapped
    vbh = np.empty((N_CORES, 128, K, PPC), np.float32)
    vbh[:, 0:64] = vvc.transpose(0, 2, 1)[:, None, :, :]      # top: vv[p,k]
    vbh[:, 64:128] = swap.transpose(0, 2, 1)[:, None, :, :]   # bottom: swap
    bbh = np.empty((128, K), np.float32)
    bbh[0:64] = bias.T                            # [q, k]
    bbh[64:128] = bias.T

    in_maps = []
    for m in range(N_CORES):
        wrepv = np.empty((4, K, SCOLS), np.float32)
        wrepv[:, :, 0:QF] = wq
        wrepv[:, :, QF:SCOLS] = vv_lo[m][None, :, :]
        biasin = np.empty((128, PPC * K + K), np.float32)
        biasin[:, 0:PPC * K] = vbh[m].reshape(128, K * PPC)
        biasin[:, PPC * K:] = bbh
        in_maps.append({
            "xin": np.ascontiguousarray(xc[m]).astype(bf16),
            "wrepv": wrepv.astype(bf16),
            "biasin": biasin,
        })
    return in_maps


def _unmarshal_output(dev_outs):
    """dev_outs: list of 8 arrays [NPAIR, 128, 4096] -> (B, C_OUT, 512, 512)."""
    out = np.empty((B, C_OUT, HW, HW), np.float32)
    patches = np.empty((N_PATCH, C_OUT, P_SZ, P_SZ), np.float32)
    for m in range(N_CORES):
        a = dev_outs[m].astype(np.float32).reshape(NPAIR, 2, C_OUT, 4, 2, 8, P_SZ)
        # axes: j, h, co, c2, cp, yy, x ; patch_local = h ^ cp
        p0 = a[:, :, :, :, 0]                      # cp=0: patch = h
        p1 = a[:, ::-1, :, :, 1]                   # cp=1: patch = 1-h
        b = np.stack([p0, p1], axis=4)             # j, patch, co, c2, cp, yy, x
        b = b.reshape(NPAIR, 2, C_OUT, NCHUNK * 8, P_SZ)  # y = (c2, cp, yy)
        patches[m * PPC:(m + 1) * PPC] = b.reshape(PPC, C_OUT, P_SZ, P_SZ)
    pt = patches.reshape(B, GRID, GRID, C_OUT, P_SZ, P_SZ)
    out = pt.transpose(0, 3, 1, 4, 2, 5).reshape(B, C_OUT, HW, HW)
    return np.ascontiguousarray(out)


def kernel(x, v, weight, bias, trace=False):
    from concourse.bass_utils import run_bass_kernel_spmd

    x = np.asarray(x, dtype=np.float32)
    v = np.asarray(v, dtype=np.float32)
    weight = np.asarray(weight, dtype=np.float32)
    bias = np.asarray(bias, dtype=np.float32)

    if "nc" not in _NC_CACHE:
        _NC_CACHE["nc"] = build_nc()
    nc = _NC_CACHE["nc"]

    in_maps = _marshal_inputs(x, v, weight, bias)
    res = run_bass_kernel_spmd(nc, in_maps, core_ids=list(range(N_CORES)),
                               trace=trace)
    dev_outs = [res.results[m]["out"] for m in range(N_CORES)]
    full = _unmarshal_output(dev_outs)
    kernel.last_result = res
    return full
